# revision 10
# baseline (speedup 1.0000x reference)
"""Multi-head attention (b=4, n=2048, dim=768, 12 heads) on 8 TRN2 NeuronCores.

Sharding: core c handles batch c//2 and head-group c%2 (6 of 12 heads).  Each
core computes its heads' contribution projected through its slice of Wo and
returns a partial [2048, 768] f32 output; the host sums core pairs and adds
the bias.  No on-device collectives needed.

Per-core kernel:
  P1: QKV projections in fp8(e4m3) DoubleRow (2 contraction chunks per pass;
      weights host-prescaled by 64, un-scaled in the PSUM->SBUF copies).
      KT/QT feature-major with head pairs stacked 64+64; V token-major in
      128-wide head blocks whose column 64 is constant 1.
  P2: scores TRANSPOSED ST[j,i] = K Q^T (K=64, bf16), exp on ACT (scale 1/8)
      for 5/6 of packed tiles and on DVE via a Schraudolph u16 bit-trick
      (f32->u16 RNE tensor_scalar, bitcast bf16) for 1/6, then
      OP[., i] += V'^T exp(ST) accumulated over j in PSUM; the ones column
      of V' makes row 64 of OP the softmax denominator l[i] for free.
      Units ordered (pair, ib, head, jc) so both heads of a pair finish
      back-to-back; exp packs 3 j-chunks per ACT/DVE instruction.
  P3: normalize without PE transposes: OP rows 0:64 are copied into a
      [128,512] pair tile (odd head shifted to partitions 64:128), 1/l rows
      are broadcast across partitions by a K=1 all-ones matmul into PSUM,
      one DVE multiply yields bf16 feature-major otn; output projection
      contracts the three 128-feature pair chunks through Wo (K=128).
"""
import os
import sys
import types
import numpy as np
import ml_dtypes

B, N, DIM = 4, 2048, 768
HEADS, DH = 12, 64
HPC = 6                # heads per core
FPC = HPC * DH         # 384 features per core
NCORES = 8
KC = DIM // 128        # 6 contraction chunks
FT = FPC // 128        # 3 feature tiles per core
NT = N // 128          # 16 key chunks of 128
IBS = 512              # i-block size
IB = N // IBS          # 4 i-blocks
BF16 = ml_dtypes.bfloat16
F8 = ml_dtypes.float8_e4m3fn
WSCALE = 64.0

EXP_A = float(0.125 * 128 / np.log(2.0))
EXP_B = float(16256 - 5.5)
DVE_EXP_MOD = 6        # exp packs with g % MOD == MOD-1 go to DVE

_cache = {}
last_exec_time_ns = None


def _install_ntff_hook():
    try:
        import antenv.axon_hooks  # noqa: F401
        return
    except ImportError:
        pass
    from trn_agent_boot.trn_boot import _ntff_profile_via_ctypes
    hook = _ntff_profile_via_ctypes('/opt/axon/libaxon_pjrt.so')
    mod = types.ModuleType('antenv.axon_hooks')
    mod.get_axon_ntff_profile_hook = lambda: hook
    import antenv
    sys.modules['antenv.axon_hooks'] = mod
    antenv.axon_hooks = mod


def _build_nc():
    from contextlib import ExitStack
    from concourse import bacc
    import concourse.mybir as mybir
    from concourse.tile import TileContext

    dt = mybir.dt
    EXP = mybir.ActivationFunctionType.Exp
    ALU = mybir.AluOpType

    nc = bacc.Bacc("TRN2", target_bir_lowering=False, debug=False,
                   num_devices=NCORES)
    xT = nc.dram_tensor("xT", [DIM, N], dt.bfloat16, kind="ExternalInput").ap()
    wq = nc.dram_tensor("wq", [DIM, FPC], dt.bfloat16, kind="ExternalInput").ap()
    wk = nc.dram_tensor("wk", [DIM, FPC], dt.bfloat16, kind="ExternalInput").ap()
    wv = nc.dram_tensor("wv", [DIM, FPC], dt.bfloat16, kind="ExternalInput").ap()
    wo = nc.dram_tensor("wo", [FPC, DIM], dt.bfloat16, kind="ExternalInput").ap()
    out = nc.dram_tensor("out", [N, DIM], dt.float32, kind="ExternalOutput").ap()

    with TileContext(nc) as tc, ExitStack() as ctx:
        const = ctx.enter_context(tc.tile_pool(name="const", bufs=1))
        onesHI = const.tile([128, 64], dt.bfloat16, tag="ohi", name="ohi")
        nc.vector.memset(onesHI[:], 1.0)

        inp = ctx.enter_context(tc.tile_pool(name="inp", bufs=1))
        xts2 = [[inp.tile([128, N // 2], dt.bfloat16, tag=f"xt{k}_{hf}",
                          name=f"xt{k}_{hf}") for hf in range(2)]
                for k in range(KC)]
        wqs = [inp.tile([128, FPC], dt.bfloat16, tag=f"wq{k}", name=f"wq{k}")
               for k in range(KC)]
        wks = [inp.tile([128, FPC], dt.bfloat16, tag=f"wk{k}", name=f"wk{k}")
               for k in range(KC)]
        wvs = [inp.tile([128, FPC], dt.bfloat16, tag=f"wv{k}", name=f"wv{k}")
               for k in range(KC)]
        wos = [inp.tile([128, DIM], dt.bfloat16, tag=f"wo{f}", name=f"wo{f}")
               for f in range(FT)]
        for k in range(KC):
            nc.sync.dma_start(out=xts2[k][0][:],
                              in_=xT[k * 128:(k + 1) * 128, 0:N // 2])
            nc.scalar.dma_start(out=wvs[k][:], in_=wv[k * 128:(k + 1) * 128, :])
        for k in range(KC):
            nc.sync.dma_start(out=xts2[k][1][:],
                              in_=xT[k * 128:(k + 1) * 128, N // 2:N])
        for k in range(KC):
            nc.sync.dma_start(out=wks[k][:], in_=wk[k * 128:(k + 1) * 128, :])
            nc.sync.dma_start(out=wqs[k][:], in_=wq[k * 128:(k + 1) * 128, :])
        for f in range(FT):
            nc.scalar.dma_start(out=wos[f][:], in_=wo[f * 128:(f + 1) * 128, :])

        kqv = ctx.enter_context(tc.tile_pool(name="kqv", bufs=1))
        KT = [kqv.tile([128, N], dt.bfloat16, tag=f"kt{f}", name=f"kt{f}")
              for f in range(FT)]
        QT = [kqv.tile([128, N], dt.bfloat16, tag=f"qt{f}", name=f"qt{f}")
              for f in range(FT)]
        VP = [kqv.tile([128, HPC * 128], dt.bfloat16, tag=f"vp{t}", name=f"vp{t}")
              for t in range(NT)]
        opsb = ctx.enter_context(tc.tile_pool(name="opsb", bufs=1))
        OPSP = [[opsb.tile([128, IBS], dt.float32, tag=f"op{p}_{ib}",
                           name=f"op{p}_{ib}") for ib in range(IB)]
                for p in range(HPC // 2)]
        otnb = ctx.enter_context(tc.tile_pool(name="otnb", bufs=1))
        OTN = [[otnb.tile([128, IBS], dt.bfloat16, tag=f"ot{p}_{ib}",
                          name=f"ot{p}_{ib}") for ib in range(IB)]
               for p in range(HPC // 2)]

        # ---- P1: fp8 DoubleRow projections ----
        for t in range(NT):
            nc.vector.memset(
                VP[t].rearrange("p (h c) -> p h c", c=128)[:, :, 64:65], 1.0)
        with tc.tile_pool(name="p1ps", bufs=3, space="PSUM") as p1:
            for t in range(NT):
                ps = p1.tile([128, FPC], dt.float32, tag="p1", name=f"vps{t}")
                for k in range(KC):
                    nc.tensor.matmul(
                        ps[:],
                        lhsT=xts2[k][t // 8][:, (t % 8) * 128:(t % 8 + 1) * 128],
                        rhs=wvs[k][:], start=(k == 0), stop=(k == KC - 1))
                nc.vector.tensor_copy(
                    VP[t].rearrange("p (h c) -> p h c", c=128)[:, :, 0:64],
                    ps.rearrange("p (h c) -> p h c", c=64))
            for W, DST in ((wks, KT), (wqs, QT)):
                for f in range(FT):
                    for q in range(N // 512):
                        ps = p1.tile([128, 512], dt.float32, tag="p1",
                                     name=f"kqps{f}_{q}")
                        for k in range(KC):
                            nc.tensor.matmul(
                                ps[:], lhsT=W[k][:, f * 128:(f + 1) * 128],
                                rhs=xts2[k][q // 2][:, (q % 2) * 512:
                                                    (q % 2 + 1) * 512],
                                start=(k == 0), stop=(k == KC - 1))
                        nc.vector.tensor_copy(DST[f][:, q * 512:(q + 1) * 512],
                                              ps[:])

        # ---- P2: attention (units ordered pair-major) + fused normalize ----
        PACK = 3
        units = [(2 * p + hh, ib, jc) for p in range(HPC // 2)
                 for ib in range(IB) for hh in range(2) for jc in range(NT)]
        assert len(units) % PACK == 0
        with tc.tile_pool(name="p2st", bufs=2, space="PSUM") as p2st, \
                tc.tile_pool(name="p2op", bufs=2, space="PSUM") as p2op, \
                tc.tile_pool(name="expp", bufs=4) as expp, \
                tc.tile_pool(name="lrowp", bufs=8) as lrowp:
            ops = {}
            lrows = {}
            pending_norm = []

            def flush_norm():
                # emit the 1/l broadcast matmul + normalize multiply for a
                # completed pair; lagged so DVE recips are done by PE time
                p, ib, _g = pending_norm.pop(0)
                lrep = p2op.tile([128, IBS], dt.float32, tag="op",
                                 name=f"lrep{p}_{ib}")
                for hh in range(2):
                    lr = lrows.pop((2 * p + hh, ib))
                    nc.tensor.matmul(
                        lrep[hh * 64:(hh + 1) * 64, :],
                        lhsT=onesHI[64:65, :],
                        rhs=lr[64:65, :],
                        start=True, stop=True)
                nc.vector.tensor_tensor(OTN[p][ib][:], OPSP[p][ib][:],
                                        lrep[:], ALU.mult)

            for g in range(len(units) // PACK):
                while pending_norm and pending_norm[0][2] <= g - 2:
                    flush_norm()
                pack = units[g * PACK:(g + 1) * PACK]
                st = p2st.tile([128, PACK * IBS], dt.float32, tag="st",
                               name=f"st{g}")
                for u, (h, ib, jc) in enumerate(pack):
                    ktf, qtf, r0 = KT[h // 2], QT[h // 2], (h % 2) * 64
                    nc.tensor.matmul(
                        st[:, u * IBS:(u + 1) * IBS],
                        lhsT=ktf[r0:r0 + 64, jc * 128:(jc + 1) * 128],
                        rhs=qtf[r0:r0 + 64, ib * IBS:(ib + 1) * IBS],
                        start=True, stop=True)
                ex = expp.tile([128, PACK * IBS], dt.bfloat16, tag="ex",
                               name=f"ex{g}")
                if g % DVE_EXP_MOD == DVE_EXP_MOD - 1:
                    nc.vector.tensor_scalar(ex[:].bitcast(dt.uint16), st[:],
                                            EXP_A, EXP_B, ALU.mult, ALU.add)
                else:
                    nc.scalar.activation(ex[:], st[:], EXP, scale=0.125)
                for u, (h, ib, jc) in enumerate(pack):
                    if jc == 0:
                        ops[(h, ib)] = p2op.tile([128, IBS], dt.float32,
                                                 tag="op", name=f"opp{h}_{ib}")
                    nc.tensor.matmul(
                        ops[(h, ib)][:], lhsT=VP[jc][:, h * 128:(h + 1) * 128],
                        rhs=ex[:, u * IBS:(u + 1) * IBS],
                        start=(jc == 0), stop=(jc == NT - 1))
                    if jc == NT - 1:
                        op = ops.pop((h, ib))
                        p, hh = divmod(h, 2)
                        # 1/l of row 64 (kept partition-aligned at row 64)
                        lr = lrowp.tile([128, IBS], dt.bfloat16, tag="lrow",
                                        name=f"lrow{h}_{ib}")
                        with nc.allow_low_precision(
                                reason="1/l rounded to bf16 for the "
                                       "broadcast matmul rhs"):
                            nc.vector.reciprocal(lr[64:65, :], op[64:65, :])
                        lrows[(h, ib)] = lr
                        # pack rows 0:64 into the pair tile (odd head shifted)
                        nc.vector.tensor_copy(
                            OPSP[p][ib][hh * 64:(hh + 1) * 64, :], op[0:64, :])
                        if hh == 1:
                            pending_norm.append((p, ib, g))
            while pending_norm:
                flush_norm()

        # ---- P3: output projection (2-stage pipeline) ----
        with tc.tile_pool(name="p3pp", bufs=4, space="PSUM") as p3pp, \
                tc.tile_pool(name="outst", bufs=3) as outst:
            for isub in range(NT):
                ib, col = isub // 4, (isub % 4) * 128
                ob = outst.tile([128, DIM], dt.float32, tag="ob",
                                name=f"ob{isub}")
                for half in range(2):
                    pp = p3pp.tile([128, DIM // 2], dt.float32, tag="pp",
                                   name=f"pp{isub}_{half}")
                    for p in range(FT):
                        nc.tensor.matmul(
                            pp[:], lhsT=OTN[p][ib][:, col:col + 128],
                            rhs=wos[p][:, half * 384:(half + 1) * 384],
                            start=(p == 0), stop=(p == FT - 1))
                    nc.scalar.copy(ob[:, half * 384:(half + 1) * 384], pp[:])
                nc.sync.dma_start(out=out[isub * 128:(isub + 1) * 128, :],
                                  in_=ob[:])

    nc.finalize()
    return nc


def _get_nc():
    if "nc" not in _cache:
        _cache["nc"] = _build_nc()
    return _cache["nc"]


def kernel(x, Wq, Wk, Wv, Wo, bo):
    global last_exec_time_ns
    x = np.asarray(x, dtype=np.float32)
    Wq = np.asarray(Wq, dtype=np.float32)
    Wk = np.asarray(Wk, dtype=np.float32)
    Wv = np.asarray(Wv, dtype=np.float32)
    Wo = np.asarray(Wo, dtype=np.float32)
    bo = np.asarray(bo, dtype=np.float32)

    trace = bool(os.environ.get("BASS_KERNEL_TRACE"))
    if trace:
        _install_ntff_hook()
        import concourse.bass_utils as bass_utils
        bass_utils.upload_artifacts = lambda tmpdir: tmpdir

    nc = _get_nc()

    in_maps = []
    for c in range(NCORES):
        bi, hg = divmod(c, 2)
        s = slice(hg * FPC, (hg + 1) * FPC)
        in_maps.append({
            "xT": np.ascontiguousarray(x[bi].T).astype(BF16),
            "wq": np.ascontiguousarray(Wq[:, s]).astype(BF16),
            "wk": np.ascontiguousarray(Wk[:, s]).astype(BF16),
            "wv": np.ascontiguousarray(Wv[:, s]).astype(BF16),
            "wo": np.ascontiguousarray(Wo[s, :]).astype(BF16),
        })

    from concourse.bass_utils import run_bass_kernel_spmd
    res = run_bass_kernel_spmd(nc, in_maps, list(range(NCORES)), trace=trace)
    last_exec_time_ns = res.exec_time_ns

    parts = [res.results[c]["out"] for c in range(NCORES)]
    full = np.empty((B, N, DIM), np.float32)
    for bi in range(B):
        full[bi] = parts[2 * bi] + parts[2 * bi + 1] + bo[None, :]
    return full


# revision 12
# speedup vs baseline: 1.0704x; 1.0704x over previous
"""Multi-head attention (b=4, n=2048, dim=768, 12 heads) on 8 TRN2 NeuronCores.

Sharding: core c handles batch c//2 and head-group c%2 (6 of 12 heads).  Each
core computes its heads' contribution projected through its slice of Wo and
returns a partial [2048, 768] f32 output; the host sums core pairs and adds
the bias.  No on-device collectives needed.

Per-core kernel:
  P1: QKV projections in fp8(e4m3) DoubleRow (2 contraction chunks per pass;
      weights host-prescaled by 64, un-scaled in the PSUM->SBUF copies).
      KT/QT feature-major with head pairs stacked 64+64; V token-major in
      128-wide head blocks whose column 64 is constant 1.
  P2: scores TRANSPOSED ST[j,i] = K Q^T (K=64, bf16), exp on ACT (scale 1/8)
      for 5/6 of packed tiles and on DVE via a Schraudolph u16 bit-trick
      (f32->u16 RNE tensor_scalar, bitcast bf16) for 1/6, then
      OP[., i] += V'^T exp(ST) accumulated over j in PSUM; the ones column
      of V' makes row 64 of OP the softmax denominator l[i] for free.
      Units ordered (pair, ib, head, jc) so both heads of a pair finish
      back-to-back; exp packs 3 j-chunks per ACT/DVE instruction.
  P3: normalize without PE transposes: OP rows 0:64 are copied into a
      [128,512] pair tile (odd head shifted to partitions 64:128), 1/l rows
      are broadcast across partitions by a K=1 all-ones matmul into PSUM,
      one DVE multiply yields bf16 feature-major otn; output projection
      contracts the three 128-feature pair chunks through Wo (K=128).
"""
import os
import sys
import types
import numpy as np
import ml_dtypes

B, N, DIM = 4, 2048, 768
HEADS, DH = 12, 64
HPC = 6                # heads per core
FPC = HPC * DH         # 384 features per core
NCORES = 8
KC = DIM // 128        # 6 contraction chunks
FT = FPC // 128        # 3 feature tiles per core
NT = N // 128          # 16 key chunks of 128
IBS = 512              # i-block size
IB = N // IBS          # 4 i-blocks
BF16 = ml_dtypes.bfloat16
F8 = ml_dtypes.float8_e4m3fn
WSCALE = 64.0

EXP_A = float(0.125 * 128 / np.log(2.0))
EXP_B = float(16256 - 5.5)
DVE_EXP_MOD = 6        # exp packs with g % MOD == MOD-1 go to DVE

_cache = {}
last_exec_time_ns = None


def _install_ntff_hook():
    try:
        import antenv.axon_hooks  # noqa: F401
        return
    except ImportError:
        pass
    from trn_agent_boot.trn_boot import _ntff_profile_via_ctypes
    hook = _ntff_profile_via_ctypes('/opt/axon/libaxon_pjrt.so')
    mod = types.ModuleType('antenv.axon_hooks')
    mod.get_axon_ntff_profile_hook = lambda: hook
    import antenv
    sys.modules['antenv.axon_hooks'] = mod
    antenv.axon_hooks = mod


def _build_nc():
    from contextlib import ExitStack
    from concourse import bacc
    import concourse.mybir as mybir
    from concourse.tile import TileContext

    dt = mybir.dt
    EXP = mybir.ActivationFunctionType.Exp
    ALU = mybir.AluOpType

    nc = bacc.Bacc("TRN2", target_bir_lowering=False, debug=False,
                   num_devices=NCORES)
    xT = nc.dram_tensor("xT", [DIM, N], dt.bfloat16, kind="ExternalInput").ap()
    wq = nc.dram_tensor("wq", [DIM, FPC], dt.bfloat16, kind="ExternalInput").ap()
    wk = nc.dram_tensor("wk", [DIM, FPC], dt.bfloat16, kind="ExternalInput").ap()
    wv = nc.dram_tensor("wv", [DIM, FPC], dt.bfloat16, kind="ExternalInput").ap()
    wo = nc.dram_tensor("wo", [FPC, DIM], dt.bfloat16, kind="ExternalInput").ap()
    out = nc.dram_tensor("out", [N, DIM], dt.float32, kind="ExternalOutput").ap()

    with TileContext(nc) as tc, ExitStack() as ctx:
        const = ctx.enter_context(tc.tile_pool(name="const", bufs=1))
        onesHI = const.tile([128, 64], dt.bfloat16, tag="ohi", name="ohi")
        nc.vector.memset(onesHI[:], 1.0)

        inp = ctx.enter_context(tc.tile_pool(name="inp", bufs=1))
        xts2 = [[inp.tile([128, N // 2], dt.bfloat16, tag=f"xt{k}_{hf}",
                          name=f"xt{k}_{hf}") for hf in range(2)]
                for k in range(KC)]
        wqs = [inp.tile([128, FPC], dt.bfloat16, tag=f"wq{k}", name=f"wq{k}")
               for k in range(KC)]
        wks = [inp.tile([128, FPC], dt.bfloat16, tag=f"wk{k}", name=f"wk{k}")
               for k in range(KC)]
        wvs = [inp.tile([128, FPC], dt.bfloat16, tag=f"wv{k}", name=f"wv{k}")
               for k in range(KC)]
        wos = [inp.tile([128, DIM], dt.bfloat16, tag=f"wo{f}", name=f"wo{f}")
               for f in range(FT)]
        for k in range(KC):
            nc.sync.dma_start(out=xts2[k][0][:],
                              in_=xT[k * 128:(k + 1) * 128, 0:N // 2])
            nc.scalar.dma_start(out=wvs[k][:], in_=wv[k * 128:(k + 1) * 128, :])
        for k in range(KC):
            nc.sync.dma_start(out=xts2[k][1][:],
                              in_=xT[k * 128:(k + 1) * 128, N // 2:N])
        for k in range(KC):
            nc.sync.dma_start(out=wks[k][:], in_=wk[k * 128:(k + 1) * 128, :])
            nc.sync.dma_start(out=wqs[k][:], in_=wq[k * 128:(k + 1) * 128, :])
        for f in range(FT):
            nc.scalar.dma_start(out=wos[f][:], in_=wo[f * 128:(f + 1) * 128, :])

        kqv = ctx.enter_context(tc.tile_pool(name="kqv", bufs=1))
        KT = [kqv.tile([128, N], dt.bfloat16, tag=f"kt{f}", name=f"kt{f}")
              for f in range(FT)]
        QT = [kqv.tile([128, N], dt.bfloat16, tag=f"qt{f}", name=f"qt{f}")
              for f in range(FT)]
        VP = [kqv.tile([128, HPC * 128], dt.bfloat16, tag=f"vp{t}", name=f"vp{t}")
              for t in range(NT)]
        opsb = ctx.enter_context(tc.tile_pool(name="opsb", bufs=1))
        OPSP = [[opsb.tile([128, IBS], dt.float32, tag=f"op{p}_{ib}",
                           name=f"op{p}_{ib}") for ib in range(IB)]
                for p in range(HPC // 2)]
        otnb = ctx.enter_context(tc.tile_pool(name="otnb", bufs=1))
        OTN = [[otnb.tile([128, IBS], dt.bfloat16, tag=f"ot{p}_{ib}",
                          name=f"ot{p}_{ib}") for ib in range(IB)]
               for p in range(HPC // 2)]
        # 1/l rows parked at 32-aligned partition bases: (h, ib) -> u = h*IB+ib
        # lives in LRT[u//4] row 32*(u%4)
        LRT = [otnb.tile([128, IBS], dt.bfloat16, tag=f"lrt{t}",
                         name=f"lrt{t}") for t in range(HPC * IB // 3)]

        # ---- P1: fp8 DoubleRow projections ----
        for t in range(NT):
            nc.vector.memset(
                VP[t].rearrange("p (h c) -> p h c", c=128)[:, :, 64:65], 1.0)
        with tc.tile_pool(name="p1ps", bufs=3, space="PSUM") as p1:
            for t in range(NT):
                ps = p1.tile([128, FPC], dt.float32, tag="p1", name=f"vps{t}")
                for k in range(KC):
                    nc.tensor.matmul(
                        ps[:],
                        lhsT=xts2[k][t // 8][:, (t % 8) * 128:(t % 8 + 1) * 128],
                        rhs=wvs[k][:], start=(k == 0), stop=(k == KC - 1))
                nc.vector.tensor_copy(
                    VP[t].rearrange("p (h c) -> p h c", c=128)[:, :, 0:64],
                    ps.rearrange("p (h c) -> p h c", c=64))
            for W, DST in ((wks, KT), (wqs, QT)):
                for f in range(FT):
                    for q in range(N // 512):
                        ps = p1.tile([128, 512], dt.float32, tag="p1",
                                     name=f"kqps{f}_{q}")
                        for k in range(KC):
                            nc.tensor.matmul(
                                ps[:], lhsT=W[k][:, f * 128:(f + 1) * 128],
                                rhs=xts2[k][q // 2][:, (q % 2) * 512:
                                                    (q % 2 + 1) * 512],
                                start=(k == 0), stop=(k == KC - 1))
                        nc.vector.tensor_copy(DST[f][:, q * 512:(q + 1) * 512],
                                              ps[:])

        # ---- P2: attention (units ordered pair-major) + fused normalize ----
        PACK = 3
        units = [(2 * p + hh, ib, jc) for p in range(HPC // 2)
                 for ib in range(IB) for hh in range(2) for jc in range(NT)]
        assert len(units) % PACK == 0
        with tc.tile_pool(name="p2st", bufs=2, space="PSUM") as p2st, \
                tc.tile_pool(name="p2op", bufs=2, space="PSUM") as p2op, \
                tc.tile_pool(name="expp", bufs=4) as expp:
            ops = {}

            for g in range(len(units) // PACK):
                pack = units[g * PACK:(g + 1) * PACK]
                st = p2st.tile([128, PACK * IBS], dt.float32, tag="st",
                               name=f"st{g}")
                for u, (h, ib, jc) in enumerate(pack):
                    ktf, qtf, r0 = KT[h // 2], QT[h // 2], (h % 2) * 64
                    nc.tensor.matmul(
                        st[:, u * IBS:(u + 1) * IBS],
                        lhsT=ktf[r0:r0 + 64, jc * 128:(jc + 1) * 128],
                        rhs=qtf[r0:r0 + 64, ib * IBS:(ib + 1) * IBS],
                        start=True, stop=True)
                ex = expp.tile([128, PACK * IBS], dt.bfloat16, tag="ex",
                               name=f"ex{g}")
                if g % DVE_EXP_MOD == DVE_EXP_MOD - 1:
                    nc.vector.tensor_scalar(ex[:].bitcast(dt.uint16), st[:],
                                            EXP_A, EXP_B, ALU.mult, ALU.add)
                else:
                    nc.scalar.activation(ex[:], st[:], EXP, scale=0.125)
                for u, (h, ib, jc) in enumerate(pack):
                    if jc == 0:
                        ops[(h, ib)] = p2op.tile([128, IBS], dt.float32,
                                                 tag="op", name=f"opp{h}_{ib}")
                    nc.tensor.matmul(
                        ops[(h, ib)][:], lhsT=VP[jc][:, h * 128:(h + 1) * 128],
                        rhs=ex[:, u * IBS:(u + 1) * IBS],
                        start=(jc == 0), stop=(jc == NT - 1))
                    if jc == NT - 1:
                        op = ops.pop((h, ib))
                        p, hh = divmod(h, 2)
                        # 1/l of row 64 parked in LRT at a 32-aligned base
                        u_ = h * IB + ib
                        lb = 32 * (u_ % 3)
                        with nc.allow_low_precision(
                                reason="1/l rounded to bf16 for the "
                                       "broadcast matmul rhs"):
                            nc.vector.reciprocal(
                                LRT[u_ // 3][lb:lb + 1, :], op[64:65, :])
                        # pack rows 0:64 into the pair tile (odd head shifted)
                        nc.vector.tensor_copy(
                            OPSP[p][ib][hh * 64:(hh + 1) * 64, :], op[0:64, :])

        # ---- P3: 1/l broadcast + normalize, then output projection ----
        with tc.tile_pool(name="p3lr", bufs=2, space="PSUM") as p3lr, \
                tc.tile_pool(name="p3pp", bufs=4, space="PSUM") as p3pp, \
                tc.tile_pool(name="outst", bufs=3) as outst:
            for ib in range(IB):
                for p in range(HPC // 2):
                    lrep = p3lr.tile([128, IBS], dt.float32, tag="lr",
                                     name=f"lrep{p}_{ib}")
                    for hh in range(2):
                        u_ = (2 * p + hh) * IB + ib
                        lb = 32 * (u_ % 3)
                        nc.tensor.matmul(
                            lrep[hh * 64:(hh + 1) * 64, :],
                            lhsT=onesHI[lb:lb + 1, :],
                            rhs=LRT[u_ // 3][lb:lb + 1, :],
                            start=True, stop=True)
                    nc.vector.tensor_tensor(OTN[p][ib][:], OPSP[p][ib][:],
                                            lrep[:], ALU.mult)
            for isub in range(NT):
                ib, col = isub // 4, (isub % 4) * 128
                ob = outst.tile([128, DIM], dt.float32, tag="ob",
                                name=f"ob{isub}")
                for half in range(2):
                    pp = p3pp.tile([128, DIM // 2], dt.float32, tag="pp",
                                   name=f"pp{isub}_{half}")
                    for p in range(FT):
                        nc.tensor.matmul(
                            pp[:], lhsT=OTN[p][ib][:, col:col + 128],
                            rhs=wos[p][:, half * 384:(half + 1) * 384],
                            start=(p == 0), stop=(p == FT - 1))
                    nc.scalar.copy(ob[:, half * 384:(half + 1) * 384], pp[:])
                nc.sync.dma_start(out=out[isub * 128:(isub + 1) * 128, :],
                                  in_=ob[:])

    nc.finalize()
    return nc


def _get_nc():
    if "nc" not in _cache:
        _cache["nc"] = _build_nc()
    return _cache["nc"]


def kernel(x, Wq, Wk, Wv, Wo, bo):
    global last_exec_time_ns
    x = np.asarray(x, dtype=np.float32)
    Wq = np.asarray(Wq, dtype=np.float32)
    Wk = np.asarray(Wk, dtype=np.float32)
    Wv = np.asarray(Wv, dtype=np.float32)
    Wo = np.asarray(Wo, dtype=np.float32)
    bo = np.asarray(bo, dtype=np.float32)

    trace = bool(os.environ.get("BASS_KERNEL_TRACE"))
    if trace:
        _install_ntff_hook()
        import concourse.bass_utils as bass_utils
        bass_utils.upload_artifacts = lambda tmpdir: tmpdir

    nc = _get_nc()

    in_maps = []
    for c in range(NCORES):
        bi, hg = divmod(c, 2)
        s = slice(hg * FPC, (hg + 1) * FPC)
        in_maps.append({
            "xT": np.ascontiguousarray(x[bi].T).astype(BF16),
            "wq": np.ascontiguousarray(Wq[:, s]).astype(BF16),
            "wk": np.ascontiguousarray(Wk[:, s]).astype(BF16),
            "wv": np.ascontiguousarray(Wv[:, s]).astype(BF16),
            "wo": np.ascontiguousarray(Wo[s, :]).astype(BF16),
        })

    from concourse.bass_utils import run_bass_kernel_spmd
    res = run_bass_kernel_spmd(nc, in_maps, list(range(NCORES)), trace=trace)
    last_exec_time_ns = res.exec_time_ns

    parts = [res.results[c]["out"] for c in range(NCORES)]
    full = np.empty((B, N, DIM), np.float32)
    for bi in range(B):
        full[bi] = parts[2 * bi] + parts[2 * bi + 1] + bo[None, :]
    return full


# revision 13
# speedup vs baseline: 1.1272x; 1.0531x over previous
"""Multi-head attention (b=4, n=2048, dim=768, 12 heads) on 8 TRN2 NeuronCores.

Sharding: core c handles batch c//2 and head-group c%2 (6 of 12 heads).  Each
core computes its heads' contribution projected through its slice of Wo and
returns a partial [2048, 768] f32 output; the host sums core pairs and adds
the bias.  No on-device collectives needed.

Per-core kernel:
  P1: QKV projections in fp8(e4m3) DoubleRow (2 contraction chunks per pass;
      weights host-prescaled by 64, un-scaled in the PSUM->SBUF copies).
      KT/QT feature-major with head pairs stacked 64+64; V token-major in
      128-wide head blocks whose column 64 is constant 1.
  P2: scores TRANSPOSED ST[j,i] = K Q^T (K=64, bf16), exp on ACT (scale 1/8)
      for 5/6 of packed tiles and on DVE via a Schraudolph u16 bit-trick
      (f32->u16 RNE tensor_scalar, bitcast bf16) for 1/6, then
      OP[., i] += V'^T exp(ST) accumulated over j in PSUM; the ones column
      of V' makes row 64 of OP the softmax denominator l[i] for free.
      Units ordered (pair, ib, head, jc) so both heads of a pair finish
      back-to-back; exp packs 3 j-chunks per ACT/DVE instruction.
  P3: normalize without PE transposes: OP rows 0:64 are copied into a
      [128,512] pair tile (odd head shifted to partitions 64:128), 1/l rows
      are broadcast across partitions by a K=1 all-ones matmul into PSUM,
      one DVE multiply yields bf16 feature-major otn; output projection
      contracts the three 128-feature pair chunks through Wo (K=128).
"""
import os
import sys
import types
import numpy as np
import ml_dtypes

B, N, DIM = 4, 2048, 768
HEADS, DH = 12, 64
HPC = 6                # heads per core
FPC = HPC * DH         # 384 features per core
NCORES = 8
KC = DIM // 128        # 6 contraction chunks
FT = FPC // 128        # 3 feature tiles per core
NT = N // 128          # 16 key chunks of 128
IBS = 512              # i-block size
IB = N // IBS          # 4 i-blocks
BF16 = ml_dtypes.bfloat16
F8 = ml_dtypes.float8_e4m3fn
WSCALE = 64.0

EXP_A = float(0.125 * 128 / np.log(2.0))
EXP_B = float(16256 - 5.5)
DVE_EXP_MOD = 10**9        # exp packs with g % MOD == MOD-1 go to DVE

_cache = {}
last_exec_time_ns = None


def _install_ntff_hook():
    try:
        import antenv.axon_hooks  # noqa: F401
        return
    except ImportError:
        pass
    from trn_agent_boot.trn_boot import _ntff_profile_via_ctypes
    hook = _ntff_profile_via_ctypes('/opt/axon/libaxon_pjrt.so')
    mod = types.ModuleType('antenv.axon_hooks')
    mod.get_axon_ntff_profile_hook = lambda: hook
    import antenv
    sys.modules['antenv.axon_hooks'] = mod
    antenv.axon_hooks = mod


def _build_nc():
    from contextlib import ExitStack
    from concourse import bacc
    import concourse.mybir as mybir
    from concourse.tile import TileContext

    dt = mybir.dt
    EXP = mybir.ActivationFunctionType.Exp
    ALU = mybir.AluOpType

    nc = bacc.Bacc("TRN2", target_bir_lowering=False, debug=False,
                   num_devices=NCORES)
    xT = nc.dram_tensor("xT", [DIM, N], dt.bfloat16, kind="ExternalInput").ap()
    wq = nc.dram_tensor("wq", [DIM, FPC], dt.bfloat16, kind="ExternalInput").ap()
    wk = nc.dram_tensor("wk", [DIM, FPC], dt.bfloat16, kind="ExternalInput").ap()
    wv = nc.dram_tensor("wv", [DIM, FPC], dt.bfloat16, kind="ExternalInput").ap()
    wo = nc.dram_tensor("wo", [FPC, DIM], dt.bfloat16, kind="ExternalInput").ap()
    out = nc.dram_tensor("out", [N, DIM], dt.float32, kind="ExternalOutput").ap()

    with TileContext(nc) as tc, ExitStack() as ctx:
        const = ctx.enter_context(tc.tile_pool(name="const", bufs=1))
        onesHI = const.tile([128, 64], dt.bfloat16, tag="ohi", name="ohi")
        nc.vector.memset(onesHI[:], 1.0)

        inp = ctx.enter_context(tc.tile_pool(name="inp", bufs=1))
        xts2 = [[inp.tile([128, N // 2], dt.bfloat16, tag=f"xt{k}_{hf}",
                          name=f"xt{k}_{hf}") for hf in range(2)]
                for k in range(KC)]
        wqs = [inp.tile([128, FPC], dt.bfloat16, tag=f"wq{k}", name=f"wq{k}")
               for k in range(KC)]
        wks = [inp.tile([128, FPC], dt.bfloat16, tag=f"wk{k}", name=f"wk{k}")
               for k in range(KC)]
        wvs = [inp.tile([128, FPC], dt.bfloat16, tag=f"wv{k}", name=f"wv{k}")
               for k in range(KC)]
        wos = [inp.tile([128, DIM], dt.bfloat16, tag=f"wo{f}", name=f"wo{f}")
               for f in range(FT)]
        for k in range(KC):
            nc.sync.dma_start(out=xts2[k][0][:],
                              in_=xT[k * 128:(k + 1) * 128, 0:N // 2])
            nc.scalar.dma_start(out=wvs[k][:], in_=wv[k * 128:(k + 1) * 128, :])
        for k in range(KC):
            nc.sync.dma_start(out=xts2[k][1][:],
                              in_=xT[k * 128:(k + 1) * 128, N // 2:N])
        for k in range(KC):
            nc.sync.dma_start(out=wks[k][:], in_=wk[k * 128:(k + 1) * 128, :])
            nc.sync.dma_start(out=wqs[k][:], in_=wq[k * 128:(k + 1) * 128, :])
        for f in range(FT):
            nc.scalar.dma_start(out=wos[f][:], in_=wo[f * 128:(f + 1) * 128, :])

        kqv = ctx.enter_context(tc.tile_pool(name="kqv", bufs=1))
        KT = [kqv.tile([128, N], dt.bfloat16, tag=f"kt{f}", name=f"kt{f}")
              for f in range(FT)]
        QT = [kqv.tile([128, N], dt.bfloat16, tag=f"qt{f}", name=f"qt{f}")
              for f in range(FT)]
        VP = [kqv.tile([128, HPC * 128], dt.bfloat16, tag=f"vp{t}", name=f"vp{t}")
              for t in range(NT)]
        opsb = ctx.enter_context(tc.tile_pool(name="opsb", bufs=1))
        OPSP = [[opsb.tile([128, IBS], dt.float32, tag=f"op{p}_{ib}",
                           name=f"op{p}_{ib}") for ib in range(IB)]
                for p in range(HPC // 2)]
        otnb = ctx.enter_context(tc.tile_pool(name="otnb", bufs=1))
        OTN = [[otnb.tile([128, IBS], dt.bfloat16, tag=f"ot{p}_{ib}",
                          name=f"ot{p}_{ib}") for ib in range(IB)]
               for p in range(HPC // 2)]
        # 1/l rows parked at 32-aligned partition bases: (h, ib) -> u = h*IB+ib
        # lives in LRT[u//4] row 32*(u%4)
        LRT = [otnb.tile([128, IBS], dt.bfloat16, tag=f"lrt{t}",
                         name=f"lrt{t}") for t in range(HPC * IB // 3)]

        # ---- P1: fp8 DoubleRow projections ----
        for t in range(NT):
            nc.vector.memset(
                VP[t].rearrange("p (h c) -> p h c", c=128)[:, :, 64:65], 1.0)
        with tc.tile_pool(name="p1ps", bufs=3, space="PSUM") as p1:
            for t in range(NT):
                ps = p1.tile([128, FPC], dt.float32, tag="p1", name=f"vps{t}")
                for k in range(KC):
                    nc.tensor.matmul(
                        ps[:],
                        lhsT=xts2[k][t // 8][:, (t % 8) * 128:(t % 8 + 1) * 128],
                        rhs=wvs[k][:], start=(k == 0), stop=(k == KC - 1))
                nc.vector.tensor_copy(
                    VP[t].rearrange("p (h c) -> p h c", c=128)[:, :, 0:64],
                    ps.rearrange("p (h c) -> p h c", c=64))
            for W, DST in ((wks, KT), (wqs, QT)):
                for f in range(FT):
                    for q in range(N // 512):
                        ps = p1.tile([128, 512], dt.float32, tag="p1",
                                     name=f"kqps{f}_{q}")
                        for k in range(KC):
                            nc.tensor.matmul(
                                ps[:], lhsT=W[k][:, f * 128:(f + 1) * 128],
                                rhs=xts2[k][q // 2][:, (q % 2) * 512:
                                                    (q % 2 + 1) * 512],
                                start=(k == 0), stop=(k == KC - 1))
                        nc.vector.tensor_copy(DST[f][:, q * 512:(q + 1) * 512],
                                              ps[:])

        # ---- P2: attention (units ordered pair-major) + fused normalize ----
        PACK = 3
        units = [(2 * p + hh, ib, jc) for p in range(HPC // 2)
                 for ib in range(IB) for hh in range(2) for jc in range(NT)]
        assert len(units) % PACK == 0
        with tc.tile_pool(name="p2st", bufs=2, space="PSUM") as p2st, \
                tc.tile_pool(name="p2op", bufs=2, space="PSUM") as p2op, \
                tc.tile_pool(name="expp", bufs=4) as expp:
            ops = {}

            for g in range(len(units) // PACK):
                pack = units[g * PACK:(g + 1) * PACK]
                st = p2st.tile([128, PACK * IBS], dt.float32, tag="st",
                               name=f"st{g}")
                for u, (h, ib, jc) in enumerate(pack):
                    ktf, qtf, r0 = KT[h // 2], QT[h // 2], (h % 2) * 64
                    nc.tensor.matmul(
                        st[:, u * IBS:(u + 1) * IBS],
                        lhsT=ktf[r0:r0 + 64, jc * 128:(jc + 1) * 128],
                        rhs=qtf[r0:r0 + 64, ib * IBS:(ib + 1) * IBS],
                        start=True, stop=True)
                ex = expp.tile([128, PACK * IBS], dt.bfloat16, tag="ex",
                               name=f"ex{g}")
                if g % DVE_EXP_MOD == DVE_EXP_MOD - 1:
                    nc.vector.tensor_scalar(ex[:].bitcast(dt.uint16), st[:],
                                            EXP_A, EXP_B, ALU.mult, ALU.add)
                else:
                    nc.scalar.activation(ex[:], st[:], EXP, scale=0.125)
                for u, (h, ib, jc) in enumerate(pack):
                    if jc == 0:
                        ops[(h, ib)] = p2op.tile([128, IBS], dt.float32,
                                                 tag="op", name=f"opp{h}_{ib}")
                    nc.tensor.matmul(
                        ops[(h, ib)][:], lhsT=VP[jc][:, h * 128:(h + 1) * 128],
                        rhs=ex[:, u * IBS:(u + 1) * IBS],
                        start=(jc == 0), stop=(jc == NT - 1))
                    if jc == NT - 1:
                        op = ops.pop((h, ib))
                        p, hh = divmod(h, 2)
                        # 1/l of row 64 parked in LRT at a 32-aligned base
                        u_ = h * IB + ib
                        lb = 32 * (u_ % 3)
                        with nc.allow_low_precision(
                                reason="1/l rounded to bf16 for the "
                                       "broadcast matmul rhs"):
                            nc.vector.reciprocal(
                                LRT[u_ // 3][lb:lb + 1, :], op[64:65, :])
                        # pack rows 0:64 into the pair tile (odd head shifted)
                        nc.vector.tensor_copy(
                            OPSP[p][ib][hh * 64:(hh + 1) * 64, :], op[0:64, :])

        # ---- P3: 1/l broadcast + normalize, then output projection ----
        with tc.tile_pool(name="p3lr", bufs=2, space="PSUM") as p3lr, \
                tc.tile_pool(name="p3pp", bufs=4, space="PSUM") as p3pp, \
                tc.tile_pool(name="outst", bufs=3) as outst:
            for ib in range(IB):
                for p in range(HPC // 2):
                    lrep = p3lr.tile([128, IBS], dt.float32, tag="lr",
                                     name=f"lrep{p}_{ib}")
                    for hh in range(2):
                        u_ = (2 * p + hh) * IB + ib
                        lb = 32 * (u_ % 3)
                        nc.tensor.matmul(
                            lrep[hh * 64:(hh + 1) * 64, :],
                            lhsT=onesHI[lb:lb + 1, :],
                            rhs=LRT[u_ // 3][lb:lb + 1, :],
                            start=True, stop=True)
                    nc.vector.tensor_tensor(OTN[p][ib][:], OPSP[p][ib][:],
                                            lrep[:], ALU.mult)
            for isub in range(NT):
                ib, col = isub // 4, (isub % 4) * 128
                ob = outst.tile([128, DIM], dt.float32, tag="ob",
                                name=f"ob{isub}")
                for half in range(2):
                    pp = p3pp.tile([128, DIM // 2], dt.float32, tag="pp",
                                   name=f"pp{isub}_{half}")
                    for p in range(FT):
                        nc.tensor.matmul(
                            pp[:], lhsT=OTN[p][ib][:, col:col + 128],
                            rhs=wos[p][:, half * 384:(half + 1) * 384],
                            start=(p == 0), stop=(p == FT - 1))
                    nc.scalar.copy(ob[:, half * 384:(half + 1) * 384], pp[:])
                nc.sync.dma_start(out=out[isub * 128:(isub + 1) * 128, :],
                                  in_=ob[:])

    nc.finalize()
    return nc


def _get_nc():
    if "nc" not in _cache:
        _cache["nc"] = _build_nc()
    return _cache["nc"]


def kernel(x, Wq, Wk, Wv, Wo, bo):
    global last_exec_time_ns
    x = np.asarray(x, dtype=np.float32)
    Wq = np.asarray(Wq, dtype=np.float32)
    Wk = np.asarray(Wk, dtype=np.float32)
    Wv = np.asarray(Wv, dtype=np.float32)
    Wo = np.asarray(Wo, dtype=np.float32)
    bo = np.asarray(bo, dtype=np.float32)

    trace = bool(os.environ.get("BASS_KERNEL_TRACE"))
    if trace:
        _install_ntff_hook()
        import concourse.bass_utils as bass_utils
        bass_utils.upload_artifacts = lambda tmpdir: tmpdir

    nc = _get_nc()

    in_maps = []
    for c in range(NCORES):
        bi, hg = divmod(c, 2)
        s = slice(hg * FPC, (hg + 1) * FPC)
        in_maps.append({
            "xT": np.ascontiguousarray(x[bi].T).astype(BF16),
            "wq": np.ascontiguousarray(Wq[:, s]).astype(BF16),
            "wk": np.ascontiguousarray(Wk[:, s]).astype(BF16),
            "wv": np.ascontiguousarray(Wv[:, s]).astype(BF16),
            "wo": np.ascontiguousarray(Wo[s, :]).astype(BF16),
        })

    from concourse.bass_utils import run_bass_kernel_spmd
    res = run_bass_kernel_spmd(nc, in_maps, list(range(NCORES)), trace=trace)
    last_exec_time_ns = res.exec_time_ns

    parts = [res.results[c]["out"] for c in range(NCORES)]
    full = np.empty((B, N, DIM), np.float32)
    for bi in range(B):
        full[bi] = parts[2 * bi] + parts[2 * bi + 1] + bo[None, :]
    return full
